# revision 18
# baseline (speedup 1.0000x reference)
"""DiffusionConv (Chebyshev graph diffusion conv) Trainium2 kernel, 8 NeuronCores.

Math (faithful to the reference's raw reshapes):
  x0 = x.reshape(n, c*b)                         # (10000, 4096)
  for each support s: x1_s = A_s @ x0 ; z_s = A_s @ x1_s
  out[bb, nn, o] = sum_{c,m} xs_m[nn, c*64+bb] Theta[c*5+m, o] + bias[o]

Restructure via matmul associativity (A acts on rows, Theta on cols => commute):
  out = x0 Th0 + bias + sum_s A_s u_s,   u_s = x0 Th1_s + A_s (x0 Th2_s)
with Th0 = m0-m2-m4, Th1_s = m_{1,3}, Th2_s = 2*m_{2,4} (m_i = Theta[:,i,:]).

Sharding: batch is data-parallel; core k owns batches [8k, 8k+8) = 512 columns
(c,bl interleaved).  Device program per core:
  phase 0: per 128-row block: PE-transpose x0 block, 5 dense GEMMs
           (v_s = x0 Th1_s, w_s = x0 Th2_s, g = x0 Th0 + bias) -> HBM:
           W_s = [w_s; v_s] (2*NPAD rows), U[NPAD:2NPAD] = g.
  phase 1: pure spmm per support: U[u_s rows] = spmm(A_s + identity, W_s)
           (identity edges add v_s: u_s = A_s w_s + v_s).
  phase 2: pure spmm: out = spmm(A_0 + identity + A_1, U) where
           U = [u_0; g; u_1] (3*NPAD rows), written f32.
Each spmm: dma_gather of source rows + one-hot PE matmul segment-sum into
PSUM per 128-row destination block (S tiles built on host).
"""

import os
from contextlib import ExitStack

import numpy as np
import ml_dtypes

import concourse.bass as bass
import concourse.bacc as bacc
import concourse.tile as tile
import concourse.mybir as mybir

# ---- problem constants (hardcoded per contest rules) ----
N_NODES = 10000
N_EDGES = 320000
N_SUPPORTS = 2
C_IN = 64
C_OUT = 64
BATCH = 64
NCORES = 8
BLOC = BATCH // NCORES          # 8 batches per core
COLS = C_IN * BLOC              # 512 columns of x0 per core
NBLK = (N_NODES + 127) // 128   # 79 dest row blocks
NPAD = NBLK * 128               # 10112

GATHER_B = 1024                 # edges per dma_gather call
TPC = GATHER_B // 128           # matmul tiles per gather call

bf16 = mybir.dt.bfloat16
f32 = mybir.dt.float32
i16 = mybir.dt.int16

LAST_RESULT = {}


# --------------------------------------------------------------------------
# host-side edge preprocessing (shared across all cores)
# --------------------------------------------------------------------------
def _prep_edges(rows, cols, vals):
    """Per dest 128-block: dedup source cols into gather SLOTS (a slot's
    gathered row can feed many output rows -> multi-hot S column, summed on
    host exactly like segment_sum). Slots sorted ascending (HBM page
    locality), padded per block to x128 and globally to xGATHER_B.
    Returns (iw, s8, tpb):
      iw: [ncalls, 128, GATHER_B//16] int16  dma_gather slot indices
      s8: [ncalls, 128, TPC*128] bf16        S^T tiles, S[slot, r] = sum v_e
    """
    rows = np.asarray(rows, np.int64)
    cols = np.asarray(cols, np.int64)
    vals = np.asarray(vals, np.float32)
    blk = rows >> 7
    order = np.argsort(blk, kind="stable")
    r_s, c_s, v_s = rows[order], cols[order], vals[order]
    blk = r_s >> 7
    counts = np.bincount(blk, minlength=NBLK)
    per_block = []
    start = 0
    for I in range(NBLK):
        cnt = int(counts[I])
        sl = slice(start, start + cnt)
        start += cnt
        uc, inv = np.unique(c_s[sl], return_inverse=True)
        nsl = len(uc)
        npad = nsl + ((-nsl) % 128)
        S_blk = np.zeros((npad, 128), np.float32)
        np.add.at(S_blk, (inv, r_s[sl] - I * 128), v_s[sl])
        per_block.append((np.concatenate([uc, np.zeros(npad - nsl,
                                                       np.int64)]), S_blk))
    return per_block


def _pack_blocks(per_block, block_ids):
    """Pack a subset of dest blocks into gather calls + S tiles + schedule."""
    idx_p, s_p, tpb = [], [], []
    for I in block_ids:
        uc, S_blk = per_block[I]
        idx_p.append(uc)
        s_p.append(S_blk)
        tpb.append(len(uc) // 128)
    idx = np.concatenate(idx_p)
    s_all = np.concatenate(s_p, 0)
    padt = (-len(idx)) % GATHER_B
    if padt:
        idx = np.concatenate([idx, np.zeros(padt, np.int64)])
        s_all = np.concatenate([s_all, np.zeros((padt, 128), np.float32)], 0)
        tpb[-1] += padt // 128
    E = len(idx)
    ncalls = E // GATHER_B
    # dma_gather index wrap: within a call, unwrapped[i] = wrap[i % 16, i // 16]
    iw = idx.reshape(ncalls, GATHER_B // 16, 16).transpose(0, 2, 1)
    iw = np.tile(iw, (1, NCORES, 1)).astype(np.int16)  # replicate to 128 parts
    # per-partition-contiguous layouts: [128, ncalls * F] so a group of calls
    # is one large contiguous-per-partition DMA
    iw = np.ascontiguousarray(iw.transpose(1, 0, 2).reshape(
        128, ncalls * (GATHER_B // 16)))
    s8 = (s_all.reshape(ncalls, TPC, 128, 128).transpose(2, 0, 1, 3)
          .reshape(128, ncalls * TPC * 128).astype(ml_dtypes.bfloat16))
    sched = []
    for I, T in zip(block_ids, tpb):
        for j in range(T):
            sched.append((I, j == 0, j == T - 1))
    return np.ascontiguousarray(iw), np.ascontiguousarray(s8), sched


def _prep_theta(Theta, bias):
    """Block-diagonal (over the 8 per-core batches) K-chunks of the five
    folded 64x64 projection matrices, order [v_0, w_0, v_1, w_1, g]."""
    Theta = np.asarray(Theta, np.float64).reshape(C_IN, 5, C_OUT)  # [c, m, co]
    m = [Theta[:, i, :] for i in range(5)]
    th_list = [m[1], 2 * m[2], m[3], 2 * m[4], m[0] - m[2] - m[4]]
    r = np.arange(128)
    j = np.arange(COLS)
    mask = (r[:, None] % BLOC) == (j[None, :] % BLOC)
    thbd = np.zeros((5, 4, 128, COLS), np.float64)
    for t, th in enumerate(th_list):
        for k in range(4):
            thbd[t, k] = th[16 * k + r[:, None] // BLOC, j[None, :] // BLOC] * mask
    bias_bd = np.repeat(np.asarray(bias, np.float64).reshape(C_OUT, 1),
                        BLOC, axis=1).reshape(1, COLS)
    c = ml_dtypes.bfloat16
    # device layout: [128 partitions, (t, k, c) free]
    thbd = np.ascontiguousarray(thbd.transpose(2, 0, 1, 3).reshape(
        128, 5 * 4 * COLS))
    return thbd.astype(c), bias_bd.astype(c)


# --------------------------------------------------------------------------
# device program (identical on all 8 cores; inputs differ per core)
# --------------------------------------------------------------------------
def _build_program(iw_shapes, scheds):
    nc = bacc.Bacc("TRN2", target_bir_lowering=False, debug=False,
                   num_swdge_queues=4, dynamic_dma_scratch_size=49152)

    # inputs
    x0b = nc.dram_tensor("x0b", [NPAD, COLS], bf16, kind="ExternalInput").ap()
    iw, s8 = [], []
    for s in range(4):  # 0: A0+id, 1: A1+id, 2/3: combined (even/odd blocks)
        iw.append(nc.dram_tensor(f"iw{s}", list(iw_shapes[s]), i16,
                                 kind="ExternalInput").ap())
        ncalls_s = iw_shapes[s][1] // (GATHER_B // 16)
        s8.append(nc.dram_tensor(f"s8{s}", [128, ncalls_s * TPC * 128],
                                 bf16, kind="ExternalInput").ap())
    thbd_d = nc.dram_tensor("thbd", [128, 5 * 4 * COLS], bf16,
                            kind="ExternalInput").ap()
    biasbd_d = nc.dram_tensor("biasbd", [1, COLS], bf16,
                              kind="ExternalInput").ap()
    ident_d = nc.dram_tensor("ident", [128, 128], bf16,
                             kind="ExternalInput").ap()

    # internal DRAM: W_s = [w_s; v_s], U = [u_0; g; u_1]
    W = [nc.dram_tensor(f"W{s}", [2 * NPAD, COLS], bf16).ap()
         for s in range(N_SUPPORTS)]
    U = nc.dram_tensor("U", [3 * NPAD, COLS], bf16).ap()

    # output
    out_d = nc.dram_tensor("out", [NPAD, COLS], f32, kind="ExternalOutput").ap()

    part = os.environ.get("KPART", "full")

    with tile.TileContext(nc) as tc, ExitStack() as ctx:
        const_p = ctx.enter_context(tc.tile_pool(name="const", bufs=1))
        ident_sb = const_p.tile([128, 128], bf16)
        nc.sync.dma_start(ident_sb[:], ident_d[:])
        thbd_sb = const_p.tile([128, 5 * 4 * COLS], bf16)
        nc.sync.dma_start(thbd_sb[:], thbd_d[:])
        biasbd_sb = const_p.tile([1, COLS], bf16)
        nc.sync.dma_start(biasbd_sb[:], biasbd_d[:])
        ones_sb = const_p.tile([1, 128], bf16)
        nc.vector.memset(ones_sb[:], 1.0)
        nreg = nc.gpsimd.to_reg(GATHER_B)
        thv = thbd_sb[:].rearrange("p (t k c) -> p t k c", t=5, k=4)

        x_pool = ctx.enter_context(tc.tile_pool(name="x", bufs=3))
        xT_pool = ctx.enter_context(tc.tile_pool(name="xT", bufs=2))
        w_pool = ctx.enter_context(tc.tile_pool(name="w", bufs=6))
        iw_pool = ctx.enter_context(tc.tile_pool(name="iw", bufs=3))
        g_pool = ctx.enter_context(tc.tile_pool(name="g", bufs=12))
        s_pool = ctx.enter_context(tc.tile_pool(name="s", bufs=4))
        y_pool = ctx.enter_context(tc.tile_pool(name="y", bufs=4))
        ps_tr = ctx.enter_context(tc.tile_pool(name="pstr", bufs=1,
                                               space="PSUM"))
        ps_w = ctx.enter_context(tc.tile_pool(name="psw", bufs=3, space="PSUM"))
        ps_y = ctx.enter_context(tc.tile_pool(name="psy", bufs=4, space="PSUM"))

        # ---- phase 0: transposes + 5 GEMMs per block, in two passes so the
        # W0 writes (needed by the first spmm) finish as early as possible ----
        # t -> destination rows: 0: W0[NPAD+] (v0), 1: W0[0+] (w0),
        #                        2: W1[NPAD+] (v1), 3: W1[0+] (w1), 4: U[NPAD+] (g)
        dsts = [(W[0], NPAD), (W[0], 0), (W[1], NPAD), (W[1], 0), (U, NPAD)]
        for ts in ((0, 1), (2, 3, 4)):
            for I in range(NBLK):
                xb = x_pool.tile([128, COLS], bf16, tag="xb")
                nc.sync.dma_start(xb[:], x0b[I * 128:(I + 1) * 128, :])
                xT = xT_pool.tile([128, 4 * 128], bf16, tag="xT")
                for k in range(4):
                    pt = ps_tr.tile([128, 128], bf16, tag="pt")
                    nc.tensor.transpose(pt[:], xb[:, k * 128:(k + 1) * 128],
                                        ident_sb[:])
                    nc.scalar.copy(xT[:, k * 128:(k + 1) * 128], pt[:])
                for t in ts:
                    pw = ps_w.tile([128, COLS], f32, tag="pw")
                    for k in range(4):
                        nc.tensor.matmul(pw[:], xT[:, k * 128:(k + 1) * 128],
                                         thv[:, t, k, :], start=(k == 0),
                                         stop=(k == 3 and t != 4))
                    if t == 4:
                        nc.tensor.matmul(pw[:], ones_sb[:], biasbd_sb[:],
                                         start=False, stop=True)
                    wsb = w_pool.tile([128, COLS], bf16, tag="wsb")
                    nc.vector.tensor_copy(wsb[:], pw[:])
                    dst, off = dsts[t]
                    nc.sync.dma_start(
                        dst[off + I * 128:off + (I + 1) * 128, :], wsb[:])

        # ---- pure spmm streams (call-granular steppers so independent
        # streams can be interleaved) ----
        CG = 4  # gather calls per S/iw load group

        def make_stream(src_ap, dst_ap, dst_off, iw_ap, s8_ap, sched, dt,
                        qbase, qnum):
            state = {"ps": None, "it": None, "sc": None}
            ncalls = len(sched) // TPC
            IWC = GATHER_B // 16
            SCC = TPC * 128

            def emit_call(c):
                if c % CG == 0:
                    ng = min(CG, ncalls - c)
                    it = iw_pool.tile([128, CG * IWC], i16, tag="iw",
                                      name="it")
                    nc.sync.dma_start(it[:, :ng * IWC],
                                      iw_ap[:, c * IWC:(c + ng) * IWC])
                    state["it"] = it
                    sc = s_pool.tile([128, CG * SCC], bf16, tag="S",
                                     name="sc")
                    nc.scalar.dma_start(sc[:, :ng * SCC],
                                        s8_ap[:, c * SCC:(c + ng) * SCC])
                    state["sc"] = sc
                q = c % CG
                G = g_pool.tile([128, TPC, COLS], bf16, tag="G")
                nc.gpsimd.dma_gather(G[:], src_ap,
                                     state["it"][:, q * IWC:(q + 1) * IWC],
                                     GATHER_B, nreg, COLS,
                                     queue_num=qbase + c % qnum)
                sc = state["sc"]
                for g in range(TPC):
                    I, first, last = sched[c * TPC + g]
                    if first:
                        state["ps"] = ps_y.tile([128, COLS], f32, tag="psy",
                                                name="psy")
                    nc.tensor.matmul(state["ps"][:],
                                     sc[:, q * SCC + g * 128:
                                        q * SCC + (g + 1) * 128],
                                     G[:, g, :], start=first, stop=last)
                    if last:
                        y_sb = y_pool.tile([128, COLS], dt, tag="ysb")
                        nc.vector.tensor_copy(y_sb[:], state["ps"][:])
                        nc.scalar.dma_start(
                            dst_ap[dst_off + I * 128:
                                   dst_off + (I + 1) * 128, :], y_sb[:])

            return ncalls, emit_call

        if part != "p0":
            # phase 1: u_s = A_s w_s + v_s, both supports interleaved
            nc0, emit0 = make_stream(W[0], U, 0, iw[0], s8[0], scheds[0],
                                     bf16, 0, 2)
            nc1, emit1 = make_stream(W[1], U, 2 * NPAD, iw[1], s8[1],
                                     scheds[1], bf16, 2, 2)
            for c in range(max(nc0, nc1)):
                if c < nc0:
                    emit0(c)
                if c < nc1:
                    emit1(c)
        if part not in ("p0", "p1"):
            # phase 2: out = A_0 u_0 + g + A_1 u_1, even/odd dest blocks
            # as two interleaved streams
            nc2, emit2 = make_stream(U, out_d, 0, iw[2], s8[2], scheds[2],
                                     f32, 0, 2)
            nc3, emit3 = make_stream(U, out_d, 0, iw[3], s8[3], scheds[3],
                                     f32, 2, 2)
            for c in range(max(nc2, nc3)):
                if c < nc2:
                    emit2(c)
                if c < nc3:
                    emit3(c)
    nc.compile()
    return nc


# --------------------------------------------------------------------------
# public entry point
# --------------------------------------------------------------------------
def kernel(x, edge_vals, Theta, bias, edge_rows, edge_cols):
    x = np.ascontiguousarray(np.asarray(x, np.float32))
    edge_vals = np.asarray(edge_vals, np.float32)
    edge_rows = np.asarray(edge_rows, np.int64)
    edge_cols = np.asarray(edge_cols, np.int64)

    # ---- host prep ----
    x0 = x.reshape(N_NODES, C_IN * BATCH).reshape(N_NODES, C_IN, BATCH)
    thbd, bias_bd = _prep_theta(Theta, bias)
    ident_np = np.eye(128, dtype=ml_dtypes.bfloat16)

    ids = np.arange(N_NODES, dtype=np.int64)
    ones_v = np.ones(N_NODES, np.float32)
    # level-1 per support: A_s edges + identity edges from the v_s guest rows
    ext = []
    for s in range(N_SUPPORTS):
        r = np.concatenate([edge_rows[s], ids])
        c = np.concatenate([edge_cols[s], NPAD + ids])
        v = np.concatenate([edge_vals[s], ones_v])
        ext.append((r, c, v))
    # level-2 combined: A_0 over u_0 (offset 0), identity over g (offset NPAD),
    # A_1 over u_1 (offset 2*NPAD)
    r2 = np.concatenate([edge_rows[0], ids, edge_rows[1]])
    c2 = np.concatenate([edge_cols[0], NPAD + ids, 2 * NPAD + edge_cols[1]])
    v2 = np.concatenate([edge_vals[0], ones_v, edge_vals[1]])
    ext.append((r2, c2, v2))

    all_blocks = list(range(NBLK))
    packs = [
        _pack_blocks(_prep_edges(*ext[0]), all_blocks),
        _pack_blocks(_prep_edges(*ext[1]), all_blocks),
    ]
    pb2 = _prep_edges(*ext[2])
    packs.append(_pack_blocks(pb2, all_blocks[0::2]))
    packs.append(_pack_blocks(pb2, all_blocks[1::2]))
    iw_s = [p[0] for p in packs]
    s8_s = [p[1] for p in packs]
    sched_s = [p[2] for p in packs]

    nc = _build_program([a.shape for a in iw_s], sched_s)

    in_maps = []
    for k in range(NCORES):
        xk = x0[:, :, k * BLOC:(k + 1) * BLOC].reshape(N_NODES, COLS)
        xkp = np.zeros((NPAD, COLS), np.float32)
        xkp[:N_NODES] = xk
        im = {"x0b": np.asarray(xkp.astype(ml_dtypes.bfloat16)),
              "thbd": np.asarray(thbd), "biasbd": np.asarray(bias_bd),
              "ident": np.asarray(ident_np)}
        for s in range(4):
            im[f"iw{s}"] = iw_s[s]
            im[f"s8{s}"] = s8_s[s]
        in_maps.append(im)

    results = _run_pjrt(nc, in_maps)

    # ---- host assembly ----
    out = np.empty((BATCH, N_NODES, C_OUT), np.float32)
    for k in range(NCORES):
        ok = results[k]["out"][:N_NODES]          # [10000, COLS] f32
        ok = ok.reshape(N_NODES, C_OUT, BLOC)
        out[k * BLOC:(k + 1) * BLOC] = np.transpose(ok, (2, 0, 1))
    return out


# --------------------------------------------------------------------------
# PJRT execution (axon) — vendored from bass2jax.run_bass_via_pjrt, but
# without output-buffer donation so the compiled executable can be
# re-dispatched for timing (our kernel fully writes its output tensor).
# --------------------------------------------------------------------------
def _run_pjrt(nc, in_maps):
    import jax
    from jax.sharding import Mesh, PartitionSpec, NamedSharding
    from jax.experimental.shard_map import shard_map
    from concourse import bass2jax
    from concourse import mybir as mb

    bass2jax.install_neuronx_cc_hook()
    n_cores = len(in_maps)
    partition_name = (nc.partition_id_tensor.name
                      if nc.partition_id_tensor else None)

    in_names, out_names, out_avals, zero_outs = [], [], [], []
    for alloc in nc.m.functions[0].allocations:
        if not isinstance(alloc, mb.MemoryLocationSet):
            continue
        name = alloc.memorylocations[0].name
        if alloc.kind == "ExternalInput":
            if name != partition_name:
                in_names.append(name)
        elif alloc.kind == "ExternalOutput":
            out_names.append(name)
            shape = tuple(alloc.tensor_shape)
            dtype = mb.dt.np(alloc.dtype)
            out_avals.append(jax.core.ShapedArray(shape, dtype))
            zero_outs.append(np.zeros(shape, dtype))
    n_params = len(in_names)
    in_names.extend(out_names)
    if partition_name is not None:
        in_names.append(partition_name)

    def _body(*args):
        operands = list(args)
        if partition_name is not None:
            operands.append(bass2jax.partition_id_tensor())
        outs = bass2jax._bass_exec_p.bind(
            *operands,
            out_avals=tuple(out_avals),
            in_names=tuple(in_names),
            out_names=tuple(out_names),
            lowering_input_output_aliases=(),
            sim_require_finite=True,
            sim_require_nnan=True,
            nc=nc,
        )
        return tuple(outs)

    devices = jax.devices()[:n_cores]
    mesh = Mesh(np.asarray(devices), ("core",))
    in_specs = (PartitionSpec("core"),) * (n_params + len(out_names))
    out_specs = (PartitionSpec("core"),) * len(out_names)
    sharded = jax.jit(
        shard_map(_body, mesh=mesh, in_specs=in_specs, out_specs=out_specs,
                  check_rep=False),
        keep_unused=True,
    )
    per_core = [[np.asarray(m[name]) for name in in_names[:n_params]]
                for m in in_maps]
    sh = NamedSharding(mesh, PartitionSpec("core"))
    concat_in = [
        jax.device_put(
            np.concatenate([per_core[c][i] for c in range(n_cores)], axis=0),
            sh)
        for i in range(n_params)
    ]
    concat_zeros = [
        jax.device_put(np.zeros((n_cores * z.shape[0], *z.shape[1:]), z.dtype),
                       sh)
        for z in zero_outs
    ]
    out_arrs = sharded(*concat_in, *concat_zeros)
    jax.block_until_ready(out_arrs)
    LAST_RESULT["runner"] = (sharded, concat_in, concat_zeros)
    return [
        {name: np.asarray(out_arrs[i]).reshape(n_cores, *out_avals[i].shape)[c]
         for i, name in enumerate(out_names)}
        for c in range(n_cores)
    ]


def time_kernel(repeats=8):
    """Per-execution device time via queued-dispatch slope (ns)."""
    import jax
    import time
    sharded, concat_in, concat_zeros = LAST_RESULT["runner"]

    def run_n(n):
        t0 = time.perf_counter()
        outs = [sharded(*concat_in, *concat_zeros) for _ in range(n)]
        jax.block_until_ready(outs)
        return time.perf_counter() - t0

    run_n(1)  # warm
    t1 = min(run_n(1) for _ in range(3))
    tn = min(run_n(1 + repeats) for _ in range(2))
    dt = (tn - t1) / repeats
    LAST_RESULT["t1_s"] = t1
    LAST_RESULT["tn_s"] = tn
    return dt * 1e9
